# revision 8
# baseline (speedup 1.0000x reference)
"""Chamfer loss kernel for Trainium2 (8 NeuronCores, SPMD).

Problem: predict_pc (B=4, 3, M=4096), gt_pc (B=4, 3, N=4096).
  loss = mean_m sqrt(min_n d2[m,n] + eps) + mean_n sqrt(min_m d2[m,n] + eps)

Sharding: 8 cores = 4 batches x 2 directions. Each core handles one
(batch, direction) pair: for its 4096 "query" points x_m it computes
  minF[m] = min_n ( |y_n|^2 - 2 x_m . y_n )
so that min_n d2[m,n] = |x_m|^2 + minF[m].  The |y|^2 term is folded into
the matmul as extra contraction rows with ones-weights.

Precision: the PE fp32 path is fp22-truncated, so instead every fp32
operand is split into 3 bf16 components (h/m/l). bf16 x bf16 products are
exact in the fp32 PSUM accumulator, so the K=30 stacked bf16 matmul
reproduces full fp32 precision (all 9 cross terms per coordinate + 3
|y|^2 rows, padded to K=32).

Reduction: per 128-row m-tile, the 4096 distance columns are processed in
two sub-rounds of 2048: 4 matmuls (N=512 each) write [128,1024] "keep" +
[128,1024] "evac" PSUM tiles; ScalarE copies "evac" to SBUF while the DVE
tensor_tensor_reduce(min, min) consumes keep (PSUM port) + copy (SBUF
port) in one pass -> 2 fresh elements/cycle/lane on the DVE. Rowmins are
chained across sub-rounds via the TTR initial-value scalar.

Host does only O(B*(M+N)) work: bf16 splits, |x|^2, sqrt, means.
"""

import numpy as np
import ml_dtypes

B = 4
M = 4096  # points per cloud (both clouds)
P = 128   # partitions per m-tile
NT = M // P  # 32 m-tiles
K = 32    # stacked contraction rows (30 used + 2 zero pad)
EPS = 1e-8

_PROGRAM = None


def _build_program():
    import concourse.bass as bass
    import concourse.mybir as mybir
    import concourse.tile as tile
    from concourse import bacc

    f32 = mybir.dt.float32
    bf16 = mybir.dt.bfloat16

    # Bacc (not raw Bass): its compile pipeline moves extra matmul waits to
    # ldweights and splits multi-wait sync into EventSemaphore preludes --
    # the Matmult/raw-ISA 64-byte encodings only fit one sync wait.
    nc = bacc.Bacc()
    w_d = nc.declare_dram_parameter("w", [K, M], bf16, isOutput=False)
    v_d = nc.declare_dram_parameter("v", [K, M], bf16, isOutput=False)
    o_d = nc.declare_dram_parameter("o", [P, NT], f32, isOutput=True)

    with tile.TileContext(nc) as tc:
        with (
            tc.tile_pool(name="inp", bufs=1) as inp_pool,
            tc.tile_pool(name="work", bufs=3) as work_pool,
            tc.tile_pool(name="acc", bufs=1) as acc_pool,
            tc.tile_pool(name="ps", bufs=2, space=bass.MemorySpace.PSUM) as ps_pool,
        ):
            w_s = inp_pool.tile([K, M], bf16)
            v_s = inp_pool.tile([K, M], bf16)
            nc.sync.dma_start(w_s[:], w_d[:])
            nc.sync.dma_start(v_s[:], v_d[:])

            minbuf = acc_pool.tile([P, NT], f32)
            part = acc_pool.tile([P, NT, 2], f32)

            for mt in range(NT):
                wt = w_s[:, mt * P:(mt + 1) * P]
                for s in range(2):
                    n0 = s * 2048
                    ps = ps_pool.tile([P, 2048], f32, tag="ps")
                    for j in range(4):
                        nc.tensor.matmul(ps[:, j * 512:(j + 1) * 512], wt,
                                         v_s[:, n0 + j * 512:n0 + (j + 1) * 512])
                    nc.vector.tensor_reduce(part[:, mt, s:s + 1], ps[:],
                                            axis=mybir.AxisListType.X,
                                            op=mybir.AluOpType.min)

            nc.vector.tensor_reduce(minbuf[:], part[:],
                                    axis=mybir.AxisListType.X,
                                    op=mybir.AluOpType.min)
            nc.sync.dma_start(o_d[:], minbuf[:])

    if not nc.is_finalized():
        nc.finalize()
    return nc


def _split3(x):
    """Split fp32 array into 3 bf16 components summing (in fp32) to ~x."""
    h = x.astype(ml_dtypes.bfloat16)
    r = x - h.astype(np.float32)
    m = r.astype(ml_dtypes.bfloat16)
    r2 = r - m.astype(np.float32)
    lo = r2.astype(ml_dtypes.bfloat16)
    return h, m, lo


def _build_wv(X, Y):
    """X, Y: (3, 4096) fp32. Build the K=32 stacked bf16 operands."""
    a = (-2.0 * X).astype(np.float32)
    asp = _split3(a)      # 3 x (3, M) bf16
    ysp = _split3(Y.astype(np.float32))
    y2 = np.sum(Y.astype(np.float32) * Y.astype(np.float32), axis=0,
                dtype=np.float32)  # (M,)
    y2sp = _split3(y2)

    w = np.zeros((K, M), dtype=ml_dtypes.bfloat16)
    v = np.zeros((K, M), dtype=ml_dtypes.bfloat16)
    for i in range(3):
        for j in range(3):
            r0 = (i * 3 + j) * 3
            w[r0:r0 + 3] = asp[i]
            v[r0:r0 + 3] = ysp[j]
    w[27:30] = np.ones((3, M), dtype=ml_dtypes.bfloat16)
    for j in range(3):
        v[27 + j] = y2sp[j]
    return w, v


def kernel(predict_pc, gt_pc):
    from concourse.bass_utils import run_bass_kernel_spmd

    global _PROGRAM
    if _PROGRAM is None:
        _PROGRAM = _build_program()
    nc = _PROGRAM

    p = np.asarray(predict_pc, dtype=np.float32)  # (B, 3, M)
    g = np.asarray(gt_pc, dtype=np.float32)       # (B, 3, N)

    in_maps = []
    for b in range(B):           # jobs 0..3: fwd (query = predict, search gt)
        w, v = _build_wv(p[b], g[b])
        in_maps.append({"w": w, "v": v})
    for b in range(B):           # jobs 4..7: bwd (query = gt, search predict)
        w, v = _build_wv(g[b], p[b])
        in_maps.append({"w": w, "v": v})

    res = run_bass_kernel_spmd(nc, in_maps, core_ids=list(range(2 * B)))

    fwd_elems = []
    bwd_elems = []
    for i in range(2 * B):
        o = np.asarray(res.results[i]["o"], dtype=np.float64)  # [P, NT]
        minF = o.T.reshape(M)  # row (mt*128 + m) -> o[m, mt]
        b = i % B
        X = p[b] if i < B else g[b]
        x2 = np.sum(X.astype(np.float64) ** 2, axis=0)  # (M,)
        min2 = x2 + minF
        elem = np.sqrt(np.maximum(min2, 0.0) + EPS)
        (fwd_elems if i < B else bwd_elems).append(elem)

    out = np.mean(np.concatenate(fwd_elems)) + np.mean(np.concatenate(bwd_elems))
    return np.array(out, dtype=np.float32)


# revision 9
# speedup vs baseline: 1.0328x; 1.0328x over previous
"""Chamfer loss kernel for Trainium2 (8 NeuronCores, SPMD).

Problem: predict_pc (B=4, 3, M=4096), gt_pc (B=4, 3, N=4096).
  loss = mean_m sqrt(min_n d2[m,n] + eps) + mean_n sqrt(min_m d2[m,n] + eps)

Sharding: 8 cores = 4 batches x 2 directions. Each core handles one
(batch, direction) pair: for its 4096 "query" points x_m it computes
  minF[m] = min_n ( |y_n|^2 - 2 x_m . y_n )
so that min_n d2[m,n] = |x_m|^2 + minF[m].  The |y|^2 term is folded into
the matmul as extra contraction rows with ones-weights.

Precision: the PE fp32 path is fp22-truncated, so instead every fp32
operand is split into 3 bf16 components (h/m/l). bf16 x bf16 products are
exact in the fp32 PSUM accumulator, so the K=30 stacked bf16 matmul
reproduces full fp32 precision (all 9 cross terms per coordinate + 3
|y|^2 rows, padded to K=32).

Reduction: per 128-row m-tile, the 4096 distance columns are processed in
two sub-rounds of 2048: 4 matmuls (N=512 each) write [128,1024] "keep" +
[128,1024] "evac" PSUM tiles; ScalarE copies "evac" to SBUF while the DVE
tensor_tensor_reduce(min, min) consumes keep (PSUM port) + copy (SBUF
port) in one pass -> 2 fresh elements/cycle/lane on the DVE. Rowmins are
chained across sub-rounds via the TTR initial-value scalar.

Host does only O(B*(M+N)) work: bf16 splits, |x|^2, sqrt, means.
"""

import numpy as np
import ml_dtypes

B = 4
M = 4096  # points per cloud (both clouds)
P = 128   # partitions per m-tile
NT = M // P  # 32 m-tiles
K = 32    # stacked contraction rows (30 used + 2 zero pad)
EPS = 1e-8

_PROGRAM = None


def _build_program():
    import concourse.bass as bass
    import concourse.mybir as mybir
    import concourse.tile as tile
    from concourse import bacc

    f32 = mybir.dt.float32
    bf16 = mybir.dt.bfloat16

    # Bacc (not raw Bass): its compile pipeline moves extra matmul waits to
    # ldweights and splits multi-wait sync into EventSemaphore preludes --
    # the Matmult/raw-ISA 64-byte encodings only fit one sync wait.
    nc = bacc.Bacc()
    w_d = nc.declare_dram_parameter("w", [K, M], bf16, isOutput=False)
    v_d = nc.declare_dram_parameter("v", [K, M], bf16, isOutput=False)
    o_d = nc.declare_dram_parameter("o", [P, NT], f32, isOutput=True)

    with tile.TileContext(nc) as tc:
        with (
            tc.tile_pool(name="inp", bufs=1) as inp_pool,
            tc.tile_pool(name="work", bufs=3) as work_pool,
            tc.tile_pool(name="acc", bufs=1) as acc_pool,
            tc.tile_pool(name="ps", bufs=2, space=bass.MemorySpace.PSUM) as ps_pool,
        ):
            w_s = inp_pool.tile([K, M], bf16)
            v_s = inp_pool.tile([K, M], bf16)
            # chunked input DMAs: the first m-tile's matmuls only need
            # w[:, :128] and v[:, :2048], so don't serialize the whole
            # 512KB load ahead of the pipeline.
            nc.sync.dma_start(w_s[:, 0:512], w_d[:, 0:512])
            for c in range(4):
                nc.sync.dma_start(v_s[:, c * 1024:(c + 1) * 1024],
                                  v_d[:, c * 1024:(c + 1) * 1024])
            for c in range(1, 8):
                nc.sync.dma_start(w_s[:, c * 512:(c + 1) * 512],
                                  w_d[:, c * 512:(c + 1) * 512])

            minbuf = acc_pool.tile([P, NT], f32)
            part = acc_pool.tile([P, NT, 2], f32)

            for mt in range(NT):
                wt = w_s[:, mt * P:(mt + 1) * P]
                for s in range(2):
                    n0 = s * 2048
                    ps = ps_pool.tile([P, 2048], f32, tag="ps")
                    for j in range(4):
                        nc.tensor.matmul(ps[:, j * 512:(j + 1) * 512], wt,
                                         v_s[:, n0 + j * 512:n0 + (j + 1) * 512])
                    nc.vector.tensor_reduce(part[:, mt, s:s + 1], ps[:],
                                            axis=mybir.AxisListType.X,
                                            op=mybir.AluOpType.min)

            nc.vector.tensor_reduce(minbuf[:], part[:],
                                    axis=mybir.AxisListType.X,
                                    op=mybir.AluOpType.min)
            nc.sync.dma_start(o_d[:], minbuf[:])

    if not nc.is_finalized():
        nc.finalize()
    return nc


def _split3(x):
    """Split fp32 array into 3 bf16 components summing (in fp32) to ~x."""
    h = x.astype(ml_dtypes.bfloat16)
    r = x - h.astype(np.float32)
    m = r.astype(ml_dtypes.bfloat16)
    r2 = r - m.astype(np.float32)
    lo = r2.astype(ml_dtypes.bfloat16)
    return h, m, lo


def _build_wv(X, Y):
    """X, Y: (3, 4096) fp32. Build the K=32 stacked bf16 operands."""
    a = (-2.0 * X).astype(np.float32)
    asp = _split3(a)      # 3 x (3, M) bf16
    ysp = _split3(Y.astype(np.float32))
    y2 = np.sum(Y.astype(np.float32) * Y.astype(np.float32), axis=0,
                dtype=np.float32)  # (M,)
    y2sp = _split3(y2)

    w = np.zeros((K, M), dtype=ml_dtypes.bfloat16)
    v = np.zeros((K, M), dtype=ml_dtypes.bfloat16)
    for i in range(3):
        for j in range(3):
            r0 = (i * 3 + j) * 3
            w[r0:r0 + 3] = asp[i]
            v[r0:r0 + 3] = ysp[j]
    w[27:30] = np.ones((3, M), dtype=ml_dtypes.bfloat16)
    for j in range(3):
        v[27 + j] = y2sp[j]
    return w, v


def kernel(predict_pc, gt_pc):
    from concourse.bass_utils import run_bass_kernel_spmd

    global _PROGRAM
    if _PROGRAM is None:
        _PROGRAM = _build_program()
    nc = _PROGRAM

    p = np.asarray(predict_pc, dtype=np.float32)  # (B, 3, M)
    g = np.asarray(gt_pc, dtype=np.float32)       # (B, 3, N)

    in_maps = []
    for b in range(B):           # jobs 0..3: fwd (query = predict, search gt)
        w, v = _build_wv(p[b], g[b])
        in_maps.append({"w": w, "v": v})
    for b in range(B):           # jobs 4..7: bwd (query = gt, search predict)
        w, v = _build_wv(g[b], p[b])
        in_maps.append({"w": w, "v": v})

    res = run_bass_kernel_spmd(nc, in_maps, core_ids=list(range(2 * B)))

    fwd_elems = []
    bwd_elems = []
    for i in range(2 * B):
        o = np.asarray(res.results[i]["o"], dtype=np.float64)  # [P, NT]
        minF = o.T.reshape(M)  # row (mt*128 + m) -> o[m, mt]
        b = i % B
        X = p[b] if i < B else g[b]
        x2 = np.sum(X.astype(np.float64) ** 2, axis=0)  # (M,)
        min2 = x2 + minF
        elem = np.sqrt(np.maximum(min2, 0.0) + EPS)
        (fwd_elems if i < B else bwd_elems).append(elem)

    out = np.mean(np.concatenate(fwd_elems)) + np.mean(np.concatenate(bwd_elems))
    return np.array(out, dtype=np.float32)
